# revision 38
# baseline (speedup 1.0000x reference)
"""Block-diagonal ZF equalizer (nn_BDEqualizer) as a Trainium2 Bass kernel.

Math: for every resource element (b, s, f) and UE u, solve the 8x8 complex
system H_u x_u = y_u where H_u[i, j] = h[b, 0, 8u+i, u, j, s, f] and
y_u[i] = y[b, 0, 8u+i, s, f].  Output x as [B, 1, 32, S, F, 2] (re/im last).

Strategy (data-parallel over the fft axis, per the sharding hint):
  - 8 cores, each owns a contiguous 128-subcarrier slice of F=1024.
  - The host pre-extracts the block-diagonal channel blocks AND pre-
    transposes them into the exact on-chip compute layout (subcarriers on
    the 128 SBUF partitions, the (j-plane, i-row, u/b/s) RE axes along the
    free dim), so loads and stores are plain partition-major DMAs: no
    on-chip transposes, no staging, no PSUM drains on the load path.
  - Unpivoted complex Gaussian elimination on the 9-plane augmented
    supertile, fp32 throughout, software-pipelined at two levels: within
    a step (plane k+1 is updated first, then step k+1's pivot/factors are
    computed while the bulk planes stream through the worker lanes), and
    across the two b-chunks (the second chunk's forward elimination runs
    staggered-concurrently with the first's, and the two serial back-
    substitution chains are emission-interleaved with each other, so the
    in-order engine queues never head-of-line block on a serial chain).
  - Measured (TimelineSim production cost model): 289 us/core vs 496 us
    for the single-lane DVE baseline; rel-L2 vs fp32 reference 2.9e-4.
    Balancer jitter seed picked by schedule search (seeds shift engine
    routing; candidates are only adopted when the output is bit-identical
    to the reference schedule, guarding against latent ordering hazards).
    Emission is driven per-chunk as a chain (forward gen, then back gen)
    with the two chunks round-robined, so each chunk's serial back pass
    fills the other chunk's forward-elimination queues.
  - Three elementwise lanes, balanced at build time by a greedy cost
    tracker: DVE (1.04 ns/elem), Pool (1.98 ns/elem), and the Tensor
    engine as an adder lane - identity-weight fp32 matmuls accumulate
    (H + P1 +- P2) in PSUM (weight loads are free) with ScalarE draining
    the result back to SBUF.  Complex products are emitted as interleaved
    pairs ((fr||fi) * bcast(b)), one instruction covering two planes.
"""

import os
import random

import numpy as np

import concourse.bacc as bacc
import concourse.mybir as mybir
from concourse.bass_utils import run_bass_kernel_spmd
from concourse.masks import make_identity
from concourse.tile import TileContext

B, NRX, NR, U, A, S, F = 4, 1, 32, 4, 8, 14, 1024
NCORES = 8
FS = F // NCORES        # 128 subcarriers per core
NB = 2                  # batch entries per chunk
NCH = B // NB           # chunks per core
M = U * NB * S          # 112 RE columns per chunk (u, b, s)
NP = 9                  # augmented planes: 8 matrix columns + rhs
F32 = mybir.dt.float32
AL = mybir.AluOpType

LAST_RESULTS = None     # BassKernelResults of the most recent run (for test.py)


def _off(j, i):
    """Free-dim offset of (plane j, row i) inside an H supertile."""
    return (j * A + i) * M


class _Balancer:
    """Greedy build-time engine load balancer."""

    RATE = {
        "V": 1.042 * float(os.environ.get("BD_VB", "1.1")),
        "P": 1.984 * float(os.environ.get("BD_PB", "1.05")),
    }
    FIX = {"V": 62.0, "P": 8.0}
    PE_RATE = 3 * float(os.environ.get("BD_PER", "1.8"))
    PE_FIX = float(os.environ.get("BD_PEF", "30"))
    ACT_RATE = 0.833            # drain per pair elem
    ACT_FIX = float(os.environ.get("BD_ACF", "150"))

    def __init__(self):
        self.busy = {
            "V": float(os.environ.get("BD_IV", "0")),
            "P": float(os.environ.get("BD_IP", "0")),
            "PE": float(os.environ.get("BD_IE", "0")),
            "ACT": 0.0,
        }
        self.rng = random.Random(int(os.environ.get("BD_SEED", "118")))
        self.jit = float(os.environ.get("BD_JIT", "400"))
        self.no_pool = bool(os.environ.get("BD_NO_POOL"))
        self.no_pe = bool(os.environ.get("BD_NO_PE"))

    def charge(self, eng, ns):
        self.busy[eng] += ns

    def pick_op(self, elems, cands=("V", "P")):
        if self.no_pool:
            cands = ("V",)
        best, cost = None, None
        for e in cands:
            c = self.busy[e] + elems * self.RATE[e] + self.FIX[e]
            c += self.rng.uniform(0, self.jit)
            if cost is None or c < cost:
                best, cost = e, c
        self.busy[best] = self.busy[best] + elems * self.RATE[best] + self.FIX[best]
        return best

    def pick_pair(self, elems, allow_pe=True, allow_pool=True):
        if self.no_pe:
            allow_pe = False
        vc = self.busy["V"] + 2 * (elems * self.RATE["V"] + self.FIX["V"])
        pc = self.busy["P"] + 2 * (elems * self.RATE["P"] + self.FIX["P"])
        if self.no_pool or not allow_pool:
            pc = vc + 1e9
        nchunk = (elems + 511) // 512
        pe_t = elems * self.PE_RATE + nchunk * self.PE_FIX
        act_t = elems * self.ACT_RATE + nchunk * self.ACT_FIX
        ec = max(self.busy["PE"] + pe_t, self.busy["ACT"] + act_t)
        # mixed: 2 matmul rows on PE + one PSUM-reading add on DVE
        pem_t = elems * self.PE_RATE * 2 / 3 + nchunk * self.PE_FIX
        vm_t = elems * self.RATE["V"] + nchunk * self.FIX["V"]
        mc = max(self.busy["PE"] + pem_t, self.busy["V"] + vm_t)
        if not os.environ.get("BD_MIX"):
            mc = 1e18
        if self.jit:
            vc += self.rng.uniform(0, self.jit)
            pc += self.rng.uniform(0, self.jit)
            ec += self.rng.uniform(0, self.jit)
        if allow_pe and mc < vc and mc < pc and mc < ec:
            self.busy["PE"] += pem_t
            self.busy["V"] += vm_t
            return "PM"
        if allow_pe and ec < vc and ec < pc:
            self.busy["PE"] += pe_t
            self.busy["ACT"] += act_t
            return "PE"
        if vc <= pc:
            self.busy["V"] += 2 * (elems * self.RATE["V"] + self.FIX["V"])
            return "V"
        self.busy["P"] += 2 * (elems * self.RATE["P"] + self.FIX["P"])
        return "P"


def _drive(*chains):
    """Round-robin emission across chains; each chain is a list of
    generators run in sequence (a chunk's forward then its back pass),
    so one chunk's back-substitution interleaves with the other chunk's
    still-running forward in every engine queue."""
    live = [list(c) for c in chains]
    turns = int(os.environ.get("BD_TURNS", "1"))
    alt = bool(os.environ.get("BD_ALT"))
    rnd = 0
    while any(live):
        order = live[::-1] if (alt and rnd % 2) else live
        rnd += 1
        for c in order:
            if not c:
                continue
            for _ in range(turns):
                try:
                    next(c[0])
                except StopIteration:
                    c.pop(0)
                    break


def _build():
    nc = bacc.Bacc(trn_type="TRN2")

    # Host-prepped compute-ready layouts (see _prep_core):
    #   hy_*[ci]  : [FS, NP*A*M] supertile image (planes 0..7 = H columns,
    #               plane 8 = y), free index = (j*A + i)*M + m, m = (u,b',s)
    #   out[ci,k] : [FS, 2*M] = (xr || xi) for matrix row k
    hyre = nc.dram_tensor("hy_re", [NCH, FS, NP * A * M], F32, kind="ExternalInput")
    hyim = nc.dram_tensor("hy_im", [NCH, FS, NP * A * M], F32, kind="ExternalInput")
    out = nc.dram_tensor("out", [NCH, A, FS, 2 * M], F32, kind="ExternalOutput")

    bal = _Balancer()

    with TileContext(nc) as tc:
        with (
            tc.tile_pool(name="consts", bufs=1) as consts,
            tc.tile_pool(name="supers", bufs=2) as supers,
            tc.tile_pool(name="work", bufs=1) as work,
            tc.tile_pool(name="invp", bufs=2) as invp,
            tc.tile_pool(
                name="fpool", bufs=int(os.environ.get("BD_FP", "3"))
            ) as fpool,
            tc.tile_pool(
                name="prods", bufs=int(os.environ.get("BD_PR", "3"))
            ) as prods,
            tc.tile_pool(
                name="xpool", bufs=int(os.environ.get("BD_XP", "4"))
            ) as xpool,
            tc.tile_pool(name="bpool", bufs=2) as bpool,
            tc.tile_pool(
                name="pacc", bufs=int(os.environ.get("BD_PACC", "5")),
                space="PSUM",
            ) as pacc,
        ):
            ident = consts.tile([128, 128], F32)
            make_identity(nc, ident)
            negid = consts.tile([128, 128], F32)
            nc.vector.tensor_scalar_mul(negid, ident, -1.0)

            def emul(eng, o, a, b):
                (nc.vector if eng == "V" else nc.gpsimd).tensor_mul(o, a, b)

            def pe_pair(dst, pa, pb, sa, sb, elems):
                """dst = dst (sa) pa (sb) pb via fp32 PSUM identity accumulate."""
                wa = negid if sa < 0 else ident
                wb = negid if sb < 0 else ident
                ck = int(os.environ.get("BD_CK", "512"))
                for off in range(0, elems, ck):
                    w = min(ck, elems - off)
                    ps = pacc.tile([128, 512], F32, tag="pacc")
                    nc.tensor.matmul(
                        ps[:, :w], ident, dst[:, off : off + w],
                        start=True, stop=False,
                    )
                    nc.tensor.matmul(
                        ps[:, :w], wa, pa[:, off : off + w],
                        start=False, stop=False,
                    )
                    nc.tensor.matmul(
                        ps[:, :w], wb, pb[:, off : off + w],
                        start=False, stop=True,
                    )
                    nc.scalar.copy(dst[:, off : off + w], ps[:, :w])

            def pe_mix(dst, pa, pb, sa, sb, elems):
                """T = (sa) pa (sb) pb in PSUM (2 matmuls, fresh group),
                then dst += T on DVE reading PSUM."""
                wa = negid if sa < 0 else ident
                wb = negid if sb < 0 else ident
                for off in range(0, elems, 512):
                    w = min(512, elems - off)
                    ps = pacc.tile([128, 512], F32, tag="pacc")
                    nc.tensor.matmul(
                        ps[:, :w], wa, pa[:, off : off + w],
                        start=True, stop=False,
                    )
                    nc.tensor.matmul(
                        ps[:, :w], wb, pb[:, off : off + w],
                        start=False, stop=True,
                    )
                    nc.vector.tensor_add(
                        dst[:, off : off + w], dst[:, off : off + w],
                        ps[:, :w],
                    )

            def combine(dst, pa, pb, sa, sb, elems, allow_pe=True,
                        allow_pool=True):
                """dst = dst (sa) pa (sb) pb, signs in {+1,-1}."""
                eng = bal.pick_pair(elems, allow_pe=allow_pe,
                                    allow_pool=allow_pool)
                if eng == "PM":
                    pe_mix(dst, pa, pb, sa, sb, elems)
                    return
                if eng == "PE":
                    pe_pair(dst, pa, pb, sa, sb, elems)
                    return
                ev = nc.vector if eng == "V" else nc.gpsimd
                (ev.tensor_add if sa > 0 else ev.tensor_sub)(dst, dst, pa)
                (ev.tensor_add if sb > 0 else ev.tensor_sub)(dst, dst, pb)

            # ---------------- per-chunk state + loads ----------------
            states = []
            for ci in range(NCH):
                st = {
                    "ci": ci,
                    "HRe": supers.tile(
                        [128, NP * A * M], F32, tag="HRe", name=f"HRe{ci}"
                    ),
                    "HIm": supers.tile(
                        [128, NP * A * M], F32, tag="HIm", name=f"HIm{ci}"
                    ),
                    "INV": invp.tile(
                        [128, 2 * A * M + 3 * M], F32, tag="INV",
                        name=f"INV{ci}"
                    ),
                    "fstate": {},
                }
                states.append(st)
            pieces = ((0, 1), (1, 2), (2, 5), (5, 9))
            if os.environ.get("BD_LDIL"):
                order = [(s, p) for p in pieces for s in states]
            elif os.environ.get("BD_LD2"):
                # chunk-0 prioritized, but chunk-1's early pieces pulled
                # forward so its (staggered) start is never load-blocked
                o = [(0, 0), (0, 1), (1, 0), (0, 2), (1, 1), (0, 3),
                     (1, 2), (1, 3)]
                order = [(states[s], pieces[p]) for s, p in o]
            else:
                order = [(s, p) for s in states for p in pieces]
            for st, (j0, j1) in order:
                ci = st["ci"]
                lo, hi = j0 * A * M, j1 * A * M
                nc.sync.dma_start(st["HRe"][:, lo:hi], hyre[ci, :, lo:hi])
                nc.sync.dma_start(st["HIm"][:, lo:hi], hyim[ci, :, lo:hi])

            def row(T, j, i):
                return T[:, _off(j, i) : _off(j, i) + M]

            def rows2(T, j, i0, n):
                base = _off(j, i0)
                return T[:, base : base + n * M]

            def rows3(T, j, i0, n):
                return rows2(T, j, i0, n).rearrange("p (r c) -> p r c", r=n)

            def inv_pair(st, k, n=None):
                v = st["INV"][:, : 2 * A * M].rearrange(
                    "p (j c) -> p j c", j=2
                )[:, :, k * M : (k + 1) * M]
                if n is None:
                    return v
                return v[:, :, None, :].broadcast_to([128, 2, n, M])

            def pivot_factors(st, k):
                """Pivot reciprocal + elimination factors for step k
                (critical path: pinned to DVE/ACT)."""
                HRe, HIm, INV = st["HRe"], st["HIm"], st["INV"]
                n = A - 1 - k
                a = row(HRe, k, k)
                b_ = row(HIm, k, k)
                TD = INV[:, 2 * A * M :]
                if not os.environ.get("BD_SQA"):
                    nc.vector.tensor_mul(TD[:, :M], a, a)
                    nc.vector.tensor_mul(TD[:, M : 2 * M], b_, b_)
                else:
                    nc.scalar.square(TD[:, :M], a)
                    nc.scalar.square(TD[:, M : 2 * M], b_)
                nc.vector.tensor_add(TD[:, :M], TD[:, :M], TD[:, M : 2 * M])
                nc.vector.reciprocal(TD[:, 2 * M :], TD[:, :M])
                irk = INV[:, k * M : (k + 1) * M]
                iik = INV[:, (A + k) * M : (A + k + 1) * M]
                nc.vector.tensor_mul(irk, a, TD[:, 2 * M :])
                nc.vector.tensor_mul(iik, b_, TD[:, 2 * M :])
                bal.charge("V", (4.2 * M) * bal.RATE["V"] + 4 * bal.FIX["V"])
                bal.charge("ACT", 2 * (M * bal.ACT_RATE + bal.ACT_FIX))
                if n == 0:
                    return
                car = rows3(HRe, k, k + 1, n)
                cai = rows3(HIm, k, k + 1, n)
                car4 = car[:, None, :, :].broadcast_to([128, 2, n, M])
                cai4 = cai[:, None, :, :].broadcast_to([128, 2, n, M])
                PF1 = prods.tile([128, 2 * (A - 1) * M], F32, tag="PA")
                PF2 = prods.tile([128, 2 * (A - 1) * M], F32, tag="PB")
                F2 = fpool.tile([128, 2 * (A - 1) * M], F32, tag="F2")
                p1v = PF1[:, : 2 * n * M].rearrange(
                    "p (j r c) -> p j r c", j=2, r=n
                )
                p2v = PF2[:, : 2 * n * M].rearrange(
                    "p (j r c) -> p j r c", j=2, r=n
                )
                if not os.environ.get("BD_FACNV"):
                    e1 = e2 = "V"
                    bal.charge(
                        "V", 4 * n * M * bal.RATE["V"] + 2 * bal.FIX["V"]
                    )
                else:
                    e1 = bal.pick_op(2 * n * M)
                    e2 = bal.pick_op(2 * n * M)
                emul(e1, p1v, car4, inv_pair(st, k, n))
                emul(e2, p2v, cai4, inv_pair(st, k, n))
                fre = F2[:, : n * M]
                fim = F2[:, (A - 1) * M : (A - 1) * M + n * M]
                # fre = -(a*ir + b*ii), fim = a*ii - b*ir
                nc.vector.scalar_tensor_tensor(
                    fre, PF1[:, : n * M], -1.0,
                    PF2[:, n * M : 2 * n * M],
                    AL.mult, AL.subtract,
                )
                nc.vector.tensor_sub(
                    fim, PF1[:, n * M : 2 * n * M], PF2[:, : n * M]
                )
                bal.charge("V", 2 * n * M * bal.RATE["V"] + 2 * bal.FIX["V"])
                f3 = F2[:, : 2 * (A - 1) * M].rearrange(
                    "p (j c) -> p j c", j=2
                )[:, :, : n * M]
                st["fstate"][k] = f3.rearrange("p j (r c) -> p j r c", r=n)

            def update_plane_parts(st, k, j, critical=False):
                """Eliminate column k from plane j (rows k+1..7); yields
                once between the product and combine emissions."""
                HRe, HIm = st["HRe"], st["HIm"]
                n = A - 1 - k
                f4 = st["fstate"][k]
                br = row(HRe, j, k)[:, None, None, :].broadcast_to(
                    [128, 2, n, M]
                )
                bi = row(HIm, j, k)[:, None, None, :].broadcast_to(
                    [128, 2, n, M]
                )
                PA = prods.tile([128, 2 * (A - 1) * M], F32, tag="PA")
                PB = prods.tile([128, 2 * (A - 1) * M], F32, tag="PB")
                pa4 = PA[:, : 2 * n * M].rearrange(
                    "p (j r c) -> p j r c", j=2, r=n
                )
                pb4 = PB[:, : 2 * n * M].rearrange(
                    "p (j r c) -> p j r c", j=2, r=n
                )
                crit2 = critical and bool(os.environ.get("BD_CRIT2"))
                critical = critical and bool(os.environ.get("BD_CRIT"))
                if critical or crit2:
                    ea = eb = "V"
                    bal.charge("V", 4 * n * M * bal.RATE["V"] + 2 * bal.FIX["V"])
                    if critical:
                        bal.charge("V", 4 * n * M * bal.RATE["V"])
                else:
                    ea = bal.pick_op(2 * n * M)
                    eb = bal.pick_op(2 * n * M)
                emul(ea, pa4, f4, br)
                emul(eb, pb4, f4, bi)
                yield
                # hr_j += PA[0] - PB[1];  hi_j += PB[0] + PA[1]
                hrj = rows2(HRe, j, k + 1, n)
                hij = rows2(HIm, j, k + 1, n)
                paR = PA[:, : n * M]
                paI = PA[:, n * M : 2 * n * M]
                pbR = PB[:, : n * M]
                pbI = PB[:, n * M : 2 * n * M]
                if critical:
                    nc.vector.tensor_add(hrj, hrj, paR)
                    nc.vector.tensor_sub(hrj, hrj, pbI)
                    nc.vector.tensor_add(hij, hij, pbR)
                    nc.vector.tensor_add(hij, hij, paI)
                else:
                    combine(hrj, paR, pbI, +1, -1, n * M,
                            allow_pool=not crit2)
                    combine(hij, pbR, paI, +1, +1, n * M,
                            allow_pool=not crit2)

            def solve_x(st, k):
                HRe, HIm = st["HRe"], st["HIm"]
                yr = row(HRe, 8, k)
                yi = row(HIm, 8, k)
                X2 = xpool.tile([128, 2 * M], F32, tag="X2")
                BP = bpool.tile([128, 4 * M], F32, tag="BP")
                p1 = BP[:, : 2 * M].rearrange("p (j c) -> p j c", j=2)
                p2 = BP[:, 2 * M :].rearrange("p (j c) -> p j c", j=2)
                yr2 = yr[:, None, :].broadcast_to([128, 2, M])
                yi2 = yi[:, None, :].broadcast_to([128, 2, M])
                nc.vector.tensor_mul(p1, yr2, inv_pair(st, k))
                nc.vector.tensor_mul(p2, yi2, inv_pair(st, k))
                # xr = yr*ir + yi*ii, xi = yi*ir - yr*ii
                nc.vector.tensor_add(X2[:, :M], BP[:, :M], BP[:, 3 * M :])
                nc.vector.tensor_sub(
                    X2[:, M :], BP[:, 2 * M : 3 * M], BP[:, M : 2 * M]
                )
                bal.charge("V", 6 * M * bal.RATE["V"] + 4 * bal.FIX["V"])
                return X2

            def clear_rows(st, k, X2, i0, nr, critical):
                """y_i -= H[i,k] * x_k for i = i0..i0+nr-1."""
                HRe, HIm = st["HRe"], st["HIm"]
                cr = rows3(HRe, k, i0, nr)[:, None, :, :].broadcast_to(
                    [128, 2, nr, M]
                )
                ci_ = rows3(HIm, k, i0, nr)[:, None, :, :].broadcast_to(
                    [128, 2, nr, M]
                )
                x4 = X2.rearrange("p (j c) -> p j c", j=2)[
                    :, :, None, :
                ].broadcast_to([128, 2, nr, M])
                QA = prods.tile([128, 2 * (A - 1) * M], F32, tag="PA")
                QB = prods.tile([128, 2 * (A - 1) * M], F32, tag="PB")
                qa4 = QA[:, : 2 * nr * M].rearrange(
                    "p (j r c) -> p j r c", j=2, r=nr
                )
                qb4 = QB[:, : 2 * nr * M].rearrange(
                    "p (j r c) -> p j r c", j=2, r=nr
                )
                # QA = (cr*xr, cr*xi), QB = (ci*xr, ci*xi)
                critical = critical and not os.environ.get("BD_NO_CRIT2")
                if critical:
                    ea = eb = "V"
                    bal.charge("V", 8 * nr * M * bal.RATE["V"] + 6 * bal.FIX["V"])
                else:
                    ea = bal.pick_op(2 * nr * M)
                    eb = bal.pick_op(2 * nr * M)
                emul(ea, qa4, cr, x4)
                emul(eb, qb4, ci_, x4)
                ytr = rows2(HRe, 8, i0, nr)
                yti = rows2(HIm, 8, i0, nr)
                # ytr -= QA[0] - QB[1];  yti -= QA[1] + QB[0]
                if critical:
                    nc.vector.tensor_sub(ytr, ytr, QA[:, : nr * M])
                    nc.vector.tensor_add(ytr, ytr, QB[:, nr * M : 2 * nr * M])
                    nc.vector.tensor_sub(yti, yti, QA[:, nr * M : 2 * nr * M])
                    nc.vector.tensor_sub(yti, yti, QB[:, : nr * M])
                else:
                    combine(ytr, QA[:, : nr * M],
                            QB[:, nr * M : 2 * nr * M], -1, +1, nr * M)
                    combine(yti, QA[:, nr * M : 2 * nr * M],
                            QB[:, : nr * M], -1, -1, nr * M)

            def update_plane(st, k, j, critical=False):
                for _ in update_plane_parts(st, k, j, critical):
                    pass

            def fwd_gen(st):
                fine = bool(os.environ.get("BD_FINE"))
                pivot_factors(st, 0)
                yield
                for k in range(A - 1):
                    if fine:
                        yield from update_plane_parts(st, k, k + 1, True)
                    else:
                        update_plane(st, k, k + 1, critical=True)
                    yield
                    pivot_factors(st, k + 1)
                    yield
                    for j in range(k + 2, NP):
                        if fine:
                            yield from update_plane_parts(st, k, j)
                        else:
                            update_plane(st, k, j)
                        yield

            def back_gen(st):
                ci = st["ci"]
                for k in range(A - 1, -1, -1):
                    X2 = solve_x(st, k)
                    yield
                    if k > 0:
                        clear_rows(st, k, X2, k - 1, 1, critical=True)
                        yield
                    nc.sync.dma_start(out[ci, k], X2)
                    yield
                    if k > 1:
                        clear_rows(st, k, X2, 0, k - 1, critical=False)
                        yield

            g0 = fwd_gen(states[0])
            g1 = fwd_gen(states[1])
            for _ in range(int(os.environ.get("BD_STAG", "14"))):
                try:
                    next(g0)
                except StopIteration:
                    break
            _drive(
                [g0, back_gen(states[0])],
                [g1, back_gen(states[1])],
            )

    nc.finalize()
    if os.environ.get("BD_DEBUG"):
        print("balancer busy (ns):", {k: round(v) for k, v in bal.busy.items()})
    return nc


_NC_CACHE = None


def _get_nc():
    global _NC_CACHE
    if _NC_CACHE is None:
        _NC_CACHE = _build()
    return _NC_CACHE


def _prep_core(y_re, y_im, h_re, h_im, c):
    """Host-side shard prep for core c: f-slice, block-diagonal extraction,
    and pre-transposition into the on-chip compute layout."""
    fsl = slice(c * FS, (c + 1) * FS)
    ue = np.arange(U)
    maps = {}
    for name, h, y in (("hy_re", h_re, y_re), ("hy_im", h_im, y_im)):
        h6 = h[:, 0, :, :, :, :, fsl].reshape(B, U, A, U, A, S, FS)
        hd = h6[:, ue, :, ue]                    # [u, b, i, j, s, f]
        hdt = hd.transpose(5, 3, 2, 0, 1, 4)     # [f, j, i, u, b, s]
        y5 = y[:, 0, :, :, fsl].reshape(B, U, A, S, FS)   # [b, u, i, s, f]
        yt = y5.transpose(4, 2, 1, 0, 3)         # [f, i, u, b, s]
        sup = np.empty((NCH, FS, NP, A, U, NB, S), np.float32)
        for ci in range(NCH):
            bsl = slice(ci * NB, (ci + 1) * NB)
            sup[ci, :, :A] = hdt[:, :, :, :, bsl]
            sup[ci, :, A] = yt[:, :, :, bsl]
        maps[name] = np.ascontiguousarray(sup.reshape(NCH, FS, NP * A * M))
    return maps


def kernel(y_re, y_im, h_re, h_im, **_ignored):
    global LAST_RESULTS
    y_re = np.asarray(y_re, dtype=np.float32)
    y_im = np.asarray(y_im, dtype=np.float32)
    h_re = np.asarray(h_re, dtype=np.float32)
    h_im = np.asarray(h_im, dtype=np.float32)

    nc = _get_nc()
    in_maps = [_prep_core(y_re, y_im, h_re, h_im, c) for c in range(NCORES)]
    trace = bool(int(os.environ.get("BD_TRACE", "0")))
    res = run_bass_kernel_spmd(
        nc, in_maps, core_ids=list(range(NCORES)), trace=trace
    )
    LAST_RESULTS = res
    outs = []
    for r in res.results:
        o = r["out"]                              # [ci, k, f, (c, u, b', s)]
        o = o.reshape(NCH, A, FS, 2, U, NB, S)
        o = o.transpose(0, 5, 4, 1, 6, 2, 3)      # [ci, b', u, k, s, f, c]
        o = o.reshape(B, U * A, S, FS, 2)         # [b, (u,i)=nr, s, f, c]
        outs.append(o)
    full = np.concatenate(outs, axis=3)           # [B, NR, S, F, 2]
    return np.ascontiguousarray(full[:, None])    # [B, 1, NR, S, F, 2]

